# revision 39
# baseline (speedup 1.0000x reference)
"""Trainium2 Bass kernel for nn_Backbone_3143916060887 (moe_routing).

Pipeline: 3x (5x5 conv + folded-BN + ReLU + maxpool2) -> avgpool2 -> feat
          -> router top-2 -> expert MLPs -> weighted combine -> L2 normalize.

Single fused SPMD launch on 8 cores: convs are data-parallel over batch
(16 images/core); the expert phase is expert-parallel (1 expert/core, all
128 tokens). feat + on-chip f32 router logits are AllGathered in THREE
chunks (feat y-row groups at L3 blocks b=7, b=15, and the tail) so most of
the collective latency hides under L3 compute; expert partials are summed
with a ReduceScatter back to the token-owner core, which L2-normalizes.

Conv strategy: tap-accumulation matmuls with output rows packed into PSUM
partitions, M=(y_off, Cout), K=(Cin, y_rel), kx as free-dim shifts. Maxpool
runs on DVE with cross-partition-base tensor_tensor max ops; BN bias + ReLU
are applied once per pooled value by ScalarE (bias commutes with max). The
avgpool 0.25 scale is folded into the router/expert-1 weights host-side.
Block-gather DMAs are spread across the SP/Act/Pool queues (the shared HWDGE
device otherwise gates the PE), and expert weights stream in pinned ~1.2us
quarter-loads during L2 via dummy WAW deps so the list scheduler cannot
hoist them into L1's input feed.

Fallback: if the fused build fails, a two-launch path (conv kernel + expert
kernel with host routing between) produces identical results.
"""

import numpy as np
import ml_dtypes

try:  # persistent XLA/NEFF cache so repeat processes skip the ~60s compile
    import jax
    jax.config.update("jax_compilation_cache_dir", "/tmp/jax_cache")
    jax.config.update("jax_persistent_cache_min_entry_size_bytes", -1)
    jax.config.update("jax_persistent_cache_min_compile_time_secs", 0.0)
except Exception:
    pass

import concourse.bass as bass
import concourse.bacc as bacc
import concourse.mybir as mybir
import concourse.tile as tile
from concourse.bass_utils import run_bass_kernel_spmd

NCORE = 8
B, CIN, H, W = 128, 36, 112, 112
BL = B // NCORE                      # 16 images per core
E, F, HID, D = 8, 7 * 7 * 128, 512, 256
BN_EPS = 1e-5
BF16 = ml_dtypes.bfloat16
RELU = mybir.ActivationFunctionType.Relu
MAX = mybir.AluOpType.max
ADD = mybir.AluOpType.add

# L3 K-chunking: (c, y_rel in 0..4), chunks of 22/22/20 input channels
L3_CH = [(0, 22), (22, 22), (44, 20)]


# ---------------------------------------------------------------------------
# host-side weight preparation
# ---------------------------------------------------------------------------

def _fold_bn(w, b, g, beta, m, v):
    s = g.astype(np.float64) / np.sqrt(v.astype(np.float64) + BN_EPS)
    return (w.astype(np.float64) * s[:, None, None, None],
            (b.astype(np.float64) - m.astype(np.float64)) * s + beta.astype(np.float64))


def _l1_weights(w):  # w float64 [32, 36, 5, 5] -> [96, 5, 3, 128] bf16
    out = np.zeros((96, 5, 3, 128), np.float64)
    for kx in range(5):
        for j in range(3):
            for c_r in range(12):
                for y_rel in range(8):
                    p = c_r * 8 + y_rel
                    for y_off in range(4):
                        ky = y_rel - y_off
                        if 0 <= ky < 5:
                            out[p, kx, j, y_off * 32:(y_off + 1) * 32] = \
                                w[:, j * 12 + c_r, ky, kx]
    return out.astype(BF16)


def _l2_weights(w):  # w float64 [64, 32, 5, 5] -> [96, 5, 2, 128] bf16
    out = np.zeros((96, 5, 2, 128), np.float64)
    for kx in range(5):
        for j in range(2):
            for c_r in range(16):
                for y_rel in range(6):
                    p = c_r * 6 + y_rel
                    for y_off in range(2):
                        ky = y_rel - y_off
                        if 0 <= ky < 5:
                            out[p, kx, j, y_off * 64:(y_off + 1) * 64] = \
                                w[:, j * 16 + c_r, ky, kx]
    return out.astype(BF16)


def _l3_weights(w):  # w float64 [128, 64, 5, 5] -> [110, 5, 3, 128] bf16
    out = np.zeros((110, 5, 3, 128), np.float64)
    for kx in range(5):
        for j, (coff, ccnt) in enumerate(L3_CH):
            for c_r in range(ccnt):
                for y_rel in range(5):
                    out[c_r * 5 + y_rel, kx, j, :] = w[:, coff + c_r, y_rel, kx]
    return out.astype(BF16)


def _l1_blocks(xc):
    """xc [16, 36, 112, 112] f32 -> [36, 116, 16, 116] bf16 padded (c, y, i, x).

    The (c_r, y_rel) block replication happens in the per-block DMA read
    pattern on-chip, so the host only ships the padded tensor once."""
    xpad = np.zeros((BL, 36, 116, 116), BF16)
    xpad[:, :, 2:114, 2:114] = xc.astype(BF16)
    return np.ascontiguousarray(xpad.transpose(1, 2, 0, 3))


# ---------------------------------------------------------------------------
# kernel A: conv stack -> feat  (data-parallel, 16 images/core)
# ---------------------------------------------------------------------------

def build_conv_module():
    nc = bacc.Bacc("TRN2", target_bir_lowering=False, debug=False,
                   num_devices=NCORE)
    f32, bf = mybir.dt.float32, mybir.dt.bfloat16
    xblk_d = nc.dram_tensor("xblk", [36, 116, BL, 116], bf, kind="ExternalInput")
    w1_d = nc.dram_tensor("w1t", [96, 5, 3, 128], bf, kind="ExternalInput")
    w2_d = nc.dram_tensor("w2t", [96, 5, 2, 128], bf, kind="ExternalInput")
    w3_d = nc.dram_tensor("w3t", [110, 5, 3, 128], bf, kind="ExternalInput")
    b1_d = nc.dram_tensor("b1", [128, 1], f32, kind="ExternalInput")
    b2_d = nc.dram_tensor("b2", [128, 1], f32, kind="ExternalInput")
    b3_d = nc.dram_tensor("b3", [128, 1], f32, kind="ExternalInput")
    feat_d = nc.dram_tensor("feat", [128, BL, 49], f32, kind="ExternalOutput")

    from contextlib import ExitStack
    with tile.TileContext(nc) as tc, ExitStack() as ctx:
        wp = ctx.enter_context(tc.tile_pool(name="weights", bufs=1))
        xp_pool = ctx.enter_context(tc.tile_pool(name="xblk", bufs=2))
        blkp = ctx.enter_context(tc.tile_pool(name="blk", bufs=2))
        psp = ctx.enter_context(tc.tile_pool(name="psum", bufs=4, space="PSUM"))
        vp = ctx.enter_context(tc.tile_pool(name="vtmp", bufs=4))
        pers = ctx.enter_context(tc.tile_pool(name="persist", bufs=1))

        w1t = wp.tile([96, 5, 3, 128], bf)
        w2t = wp.tile([96, 5, 2, 128], bf)
        w3t = wp.tile([110, 5, 3, 128], bf)
        b1t = wp.tile([128, 1], f32)
        b2t = wp.tile([128, 1], f32)
        b3t = wp.tile([128, 1], f32)
        for dst, src in ((w1t, w1_d), (w2t, w2_d), (w3t, w3_d),
                         (b1t, b1_d), (b2t, b2_d), (b3t, b3_d)):
            nc.sync.dma_start(dst[:], src[:])

        # persistent activations: pool1 [(q4,c32), (60 y, 4 i, 60 x)], img = q*4+i
        pool1 = pers.tile([128, 60, 4, 60], bf)
        # pool2 [(h2,c64), (32 y, 8 i, 32 x)], img = h*8+i
        pool2 = pers.tile([128, 32, 8, 32], bf)
        conv3 = pers.tile([128, BL, 28, 28], f32)
        feat_sb = pers.tile([128, BL, 7, 7], f32)
        nc.gpsimd.memset(pool1[:], 0.0)
        nc.gpsimd.memset(pool2[:], 0.0)

        # ---------------- layer 1: 36 -> 32, 112x112 -> pool 56x56
        for b in range(28):
            xt = xp_pool.tile([96, 3, BL, 116], bf)
            for j in range(3):
                nc.sync.dma_start(
                    xt[:, j].rearrange("p i x -> p (i x)"),
                    xblk_d[12 * j:12 * j + 12, 4 * b:4 * b + 8]
                    .rearrange("c y i x -> c y (i x)"))
            for sg in range(4):                      # 4 images per matmul group
                ps = psp.tile([128, 4, 112], mybir.dt.float32)
                k = 0
                for kx in range(5):
                    for j in range(3):
                        nc.tensor.matmul(
                            ps[:], w1t[:, kx, j, :],
                            xt[:, j, 4 * sg:4 * sg + 4, kx:kx + 112],
                            start=(k == 0), stop=(k == 14))
                        k += 1
                ev = vp.tile([128, 4, 112], bf)
                nc.scalar.activation(out=ev[:], in_=ps[:], func=RELU,
                                     bias=b1t[:], scale=1.0)
                # x-pool per y_off slice into base-0 tile (legal SB+SB TT
                # needs equal input bases; base-remap happens on the write)
                xpo = vp.tile([32, 4, 4, 56], bf)
                for yo in range(4):
                    nc.vector.tensor_tensor(
                        xpo[:, yo], ev[32 * yo:32 * yo + 32, :, 0::2],
                        ev[32 * yo:32 * yo + 32, :, 1::2], MAX)
                for yy in range(2):                  # pooled rows 2b, 2b+1
                    nc.vector.tensor_tensor(
                        pool1[32 * sg:32 * sg + 32, 2 + 2 * b + yy, :, 2:58],
                        xpo[:, 2 * yy], xpo[:, 2 * yy + 1], MAX)

        # ---------------- layer 2: 32 -> 64, 56x56 -> pool 28x28
        EW1_SCHED = {}
        ew1m, ew1_d = [], None
        for b in range(28):
            bt = blkp.tile([96, 2, 4, 4, 60], bf)  # p, j, q, i, x
            for j in range(2):
                for q in range(4):
                    src = pool1[32 * q + 16 * j:32 * q + 16 * j + 16,
                                2 * b:2 * b + 6, :, :]
                    nc.sync.dma_start(
                        bt[:, j, q].rearrange("p i x -> p (i x)"),
                        src.rearrange("c y i x -> c y (i x)"))
            for sg in range(2):                      # 8 images per matmul group
                ps = psp.tile([128, 2, 4, 56], mybir.dt.float32)
                k = 0
                for kx in range(5):
                    for j in range(2):
                        nc.tensor.matmul(
                            ps[:], w2t[:, kx, j, :],
                            bt[:, j, 2 * sg:2 * sg + 2, :, kx:kx + 56],
                            start=(k == 0), stop=(k == 9))
                        k += 1
                ev = vp.tile([128, 2, 4, 56], bf, tag="ev2")
                nc.scalar.activation(out=ev[:], in_=ps[:], func=RELU,
                                     bias=b2t[:], scale=1.0)
                xpo = vp.tile([64, 2, 2, 4, 28], bf, tag="xpo2")
                for yo in range(2):
                    nc.vector.tensor_tensor(
                        xpo[:, yo], ev[64 * yo:64 * yo + 64, :, :, 0::2],
                        ev[64 * yo:64 * yo + 64, :, :, 1::2], MAX)
                nc.vector.tensor_tensor(
                    pool2[64 * sg:64 * sg + 64, 2 + b, :, 2:30]
                    .rearrange("c (q i) x -> c q i x", q=2),
                    xpo[:, 0], xpo[:, 1], MAX)
            if b in EW1_SCHED:
                m, hh = EW1_SCHED[b]
                s0, s1 = EW1_Q[hh], EW1_Q[hh + 1]
                nc.vector.tensor_copy(
                    ew1m[m][0:1, s0:s0 + 1, 0:1].rearrange("c s h -> c (s h)"),
                    pool2[0:1, 1 + b:2 + b, 0:1, 2:3]
                    .rearrange("c y i x -> c (y i x)"))
                nc.sync.dma_start(ew1m[m][:, s0:s1, :], ew1_d[m, :, s0:s1, :])

        # ---------------- layer 3: 64 -> 128, 28x28 (pool later)
        for b in range(28):
            bt = blkp.tile([110, 3, 2, 8, 32], bf, tag="bt3")  # p, j, h, i, x
            for j, (coff, ccnt) in enumerate(L3_CH):
                for h in range(2):
                    src = pool2[64 * h + coff:64 * h + coff + ccnt, b:b + 5, :, :]
                    nc.sync.dma_start(
                        bt[0:ccnt * 5, j, h].rearrange("p i x -> p (i x)"),
                        src.rearrange("c y i x -> c y (i x)"))
            ps = psp.tile([128, 2, 8, 28], mybir.dt.float32)
            k = 0
            for kx in range(5):
                for j, (coff, ccnt) in enumerate(L3_CH):
                    nc.tensor.matmul(
                        ps[:], w3t[0:ccnt * 5, kx, j, :],
                        bt[0:ccnt * 5, j, :, :, kx:kx + 28],
                        start=(k == 0), stop=(k == 14))
                    k += 1
            nc.scalar.activation(
                out=conv3[:, :, b, :],
                in_=ps[:].rearrange("c h i x -> c (h i) x"),
                func=RELU, bias=b3t[:], scale=1.0)

        # ---------------- maxpool 28->14, avgpool 14->7 (per 4-image chunk)
        for i4 in range(4):
            c4 = conv3[:, 4 * i4:4 * i4 + 4]
            mx = vp.tile([128, 4, 28, 14], mybir.dt.float32, tag="mx")
            nc.vector.tensor_tensor(mx[:], c4[:, :, :, 0::2], c4[:, :, :, 1::2], MAX)
            mxy = vp.tile([128, 4, 14, 14], mybir.dt.float32, tag="mxy")
            nc.vector.tensor_tensor(mxy[:], mx[:, :, 0::2, :], mx[:, :, 1::2, :], MAX)
            ax = vp.tile([128, 4, 14, 7], mybir.dt.float32, tag="ax")
            nc.vector.tensor_tensor(ax[:], mxy[:, :, :, 0::2], mxy[:, :, :, 1::2], ADD)
            ay = vp.tile([128, 4, 7, 7], mybir.dt.float32, tag="ay")
            nc.vector.tensor_tensor(ay[:], ax[:, :, 0::2, :], ax[:, :, 1::2, :], ADD)
            nc.vector.tensor_scalar_mul(feat_sb[:, 4 * i4:4 * i4 + 4], ay[:], 0.25)

        nc.sync.dma_start(feat_d[:].rearrange("c i s -> c (i s)"),
                          feat_sb[:].rearrange("c i y x -> c (i y x)"))
    nc.compile()
    return nc


# ---------------------------------------------------------------------------
# kernel B: one expert for all 128 tokens  (expert-parallel)
# ---------------------------------------------------------------------------

def build_expert_module():
    nc = bacc.Bacc("TRN2", target_bir_lowering=False, debug=False,
                   num_devices=NCORE)
    f32, bf = mybir.dt.float32, mybir.dt.bfloat16
    ft_d = nc.dram_tensor("featT", [128, 49, 128], bf, kind="ExternalInput")
    ew1_d = nc.dram_tensor("ew1", [128, 49, 512], bf, kind="ExternalInput")
    ew2_d = nc.dram_tensor("ew2", [128, 4, 256], bf, kind="ExternalInput")
    eb1_d = nc.dram_tensor("eb1", [128, 4], f32, kind="ExternalInput")
    coef_d = nc.dram_tensor("coef", [128, 1], f32, kind="ExternalInput")
    out_d = nc.dram_tensor("partial", [128, 256], f32, kind="ExternalOutput")

    from contextlib import ExitStack
    with tile.TileContext(nc) as tc, ExitStack() as ctx:
        wp = ctx.enter_context(tc.tile_pool(name="w", bufs=1))
        psp = ctx.enter_context(tc.tile_pool(name="psum", bufs=4, space="PSUM"))
        ft = wp.tile([128, 49, 128], bf)
        ew1 = wp.tile([128, 49, 512], bf)
        ew2 = wp.tile([128, 4, 256], bf)
        eb1 = wp.tile([128, 4], f32)
        coef = wp.tile([128, 1], f32)
        hid = wp.tile([128, 4, 128], bf)
        outb = wp.tile([128, 256], f32)
        nc.sync.dma_start(ft[:], ft_d[:])
        nc.sync.dma_start(ew1[:], ew1_d[:])
        nc.sync.dma_start(ew2[:], ew2_d[:])
        nc.sync.dma_start(eb1[:], eb1_d[:])
        nc.sync.dma_start(coef[:], coef_d[:])
        for m in range(4):
            ps = psp.tile([128, 128], mybir.dt.float32)
            for s in range(49):
                nc.tensor.matmul(ps[:], ew1[:, s, 128 * m:128 * m + 128],
                                 ft[:, s, :], start=(s == 0), stop=(s == 48))
            nc.scalar.activation(out=hid[:, m, :], in_=ps[:], func=RELU,
                                 bias=eb1[:, m:m + 1], scale=1.0)
        ps2 = psp.tile([128, 256], mybir.dt.float32)
        for m in range(4):
            nc.tensor.matmul(ps2[:], hid[:, m, :], ew2[:, m, :],
                             start=(m == 0), stop=(m == 3))
        nc.vector.tensor_scalar_mul(outb[:], ps2[:], coef[:])
        nc.sync.dma_start(out_d[:], outb[:])
    nc.compile()
    return nc


# ---------------------------------------------------------------------------
# host orchestration
# ---------------------------------------------------------------------------

_CACHE = {}
LAST = {}
_TRACE = False


def set_trace(v):
    global _TRACE
    _TRACE = v


def _prep_conv_inputs(x, conv1_w, conv1_b, bn1_g, bn1_b, bn1_m, bn1_v,
                      conv2_w, conv2_b, bn2_g, bn2_b, bn2_m, bn2_v,
                      conv3_w, conv3_b, bn3_g, bn3_b, bn3_m, bn3_v):
    w1, bb1 = _fold_bn(conv1_w, conv1_b, bn1_g, bn1_b, bn1_m, bn1_v)
    w2, bb2 = _fold_bn(conv2_w, conv2_b, bn2_g, bn2_b, bn2_m, bn2_v)
    w3, bb3 = _fold_bn(conv3_w, conv3_b, bn3_g, bn3_b, bn3_m, bn3_v)
    shared = {
        "w1t": _l1_weights(w1), "w2t": _l2_weights(w2), "w3t": _l3_weights(w3),
        "b1": np.tile(bb1.astype(np.float32), 4)[:, None],
        "b2": np.tile(bb2.astype(np.float32), 2)[:, None],
        "b3": bb3.astype(np.float32)[:, None],
    }
    in_maps = []
    for c in range(NCORE):
        m = dict(shared)
        m["xblk"] = _l1_blocks(np.asarray(x[c * BL:(c + 1) * BL]))
        in_maps.append(m)
    return in_maps


def _route(feat, router_w, router_b):
    """Top-2 routing in float64. Returns coef [128, 8] (zeros off top-2)."""
    logits = feat.astype(np.float64) @ router_w.astype(np.float64).T \
        + router_b.astype(np.float64)
    order = np.argsort(-logits, axis=1, kind="stable")
    i0, i1 = order[:, 0], order[:, 1]
    t = np.arange(logits.shape[0])
    e1 = np.exp(logits[t, i1] - logits[t, i0])
    c0 = 1.0 / (1.0 + e1)
    coef = np.zeros((logits.shape[0], E), np.float64)
    coef[t, i0] = c0
    coef[t, i1] = 1.0 - c0
    return coef


import os as _os
_FUSED = _os.environ.get("KERNEL_FUSED", "1") == "1"


def kernel(**inputs):
    inputs = {k: np.asarray(v) for k, v in inputs.items()}
    if _FUSED:
        try:
            return kernel_fused(inputs)
        except Exception as e:  # fall back to the two-launch path
            import traceback
            traceback.print_exc()
            print(f"kernel_fused failed ({type(e).__name__}); "
                  "falling back to two-launch path", flush=True)
    conv_keys = ("x", "conv1_w", "conv1_b", "bn1_g", "bn1_b", "bn1_m", "bn1_v",
                 "conv2_w", "conv2_b", "bn2_g", "bn2_b", "bn2_m", "bn2_v",
                 "conv3_w", "conv3_b", "bn3_g", "bn3_b", "bn3_m", "bn3_v")
    in_maps = _prep_conv_inputs(*[inputs[k] for k in conv_keys])

    if "conv_nc" not in _CACHE:
        _CACHE["conv_nc"] = build_conv_module()
    res_a = run_bass_kernel_spmd(_CACHE["conv_nc"], in_maps,
                                 core_ids=list(range(NCORE)), trace=_TRACE)
    LAST["a"] = res_a

    # feat [128 tokens, 6272]; feat index f = c*49 + s
    feat = np.concatenate(
        [res_a.results[c]["feat"].transpose(1, 0, 2).reshape(BL, F)
         for c in range(NCORE)], axis=0).astype(np.float32)

    coef = _route(feat, inputs["router_w"], inputs["router_b"])

    featT = np.ascontiguousarray(
        feat.T.reshape(128, 49, 128).astype(BF16))          # [c, s, t]
    ew1 = inputs["ew1"]
    ew2 = inputs["ew2"]
    eb1 = inputs["eb1"]
    in_maps_b = []
    for e in range(NCORE):
        in_maps_b.append({
            "featT": featT,
            "ew1": np.ascontiguousarray(ew1[e].reshape(128, 49, 512).astype(BF16)),
            "ew2": np.ascontiguousarray(
                ew2[e].reshape(4, 128, 256).transpose(1, 0, 2).astype(BF16)),
            "eb1": np.ascontiguousarray(eb1[e].reshape(4, 128).T.astype(np.float32)),
            "coef": coef[:, e].astype(np.float32)[:, None],
        })
    if "exp_nc" not in _CACHE:
        _CACHE["exp_nc"] = build_expert_module()
    res_b = run_bass_kernel_spmd(_CACHE["exp_nc"], in_maps_b,
                                 core_ids=list(range(NCORE)), trace=_TRACE)
    LAST["b"] = res_b

    emb = sum(res_b.results[e]["partial"].astype(np.float64) for e in range(NCORE))
    emb += coef @ inputs["eb2"].astype(np.float64)           # eb2 correction
    norm = np.linalg.norm(emb, axis=1, keepdims=True)
    return (emb / np.maximum(norm, 1e-12)).astype(np.float32)


# ---------------------------------------------------------------------------
# fused single-launch module: conv -> AllGather(feat) -> router top-2 ->
# expert -> ReduceScatter -> L2 normalize
# ---------------------------------------------------------------------------

def build_fused_module():
    nc = bacc.Bacc("TRN2", target_bir_lowering=False, debug=False,
                   num_devices=NCORE)
    f32, bf = mybir.dt.float32, mybir.dt.bfloat16
    xblk_d = nc.dram_tensor("xblk", [36, 116, BL, 116], bf, kind="ExternalInput")
    w1_d = nc.dram_tensor("w1t", [96, 5, 3, 128], bf, kind="ExternalInput")
    w2_d = nc.dram_tensor("w2t", [96, 5, 2, 128], bf, kind="ExternalInput")
    w3_d = nc.dram_tensor("w3t", [110, 5, 3, 128], bf, kind="ExternalInput")
    b1_d = nc.dram_tensor("b1", [128, 1], f32, kind="ExternalInput")
    b2_d = nc.dram_tensor("b2", [128, 1], f32, kind="ExternalInput")
    b3_d = nc.dram_tensor("b3", [128, 1], f32, kind="ExternalInput")
    rwt_d = nc.dram_tensor("rwt", [128, 49, 8], f32, kind="ExternalInput")
    rb_d = nc.dram_tensor("rb", [1, 8], f32, kind="ExternalInput")
    ew1_d = nc.dram_tensor("ew1", [4, 128, 49, 128], bf, kind="ExternalInput")
    ew2_d = nc.dram_tensor("ew2", [128, 4, 256], bf, kind="ExternalInput")
    eb1_d = nc.dram_tensor("eb1", [128, 4], f32, kind="ExternalInput")
    eb2r_d = nc.dram_tensor("eb2r", [1, 256], bf, kind="ExternalInput")
    oh_d = nc.dram_tensor("oh", [128, 8], f32, kind="ExternalInput")
    y_d = nc.dram_tensor("y", [BL, 256], f32, kind="ExternalOutput")

    ag1_d = nc.dram_tensor("ag1", [128, 14, BL], bf, kind="Internal")
    cc1_d = nc.dram_tensor("cc1", [NCORE, 128, 14, BL], bf,
                           kind="Internal", addr_space="Shared")
    ag2_d = nc.dram_tensor("ag2", [128, 14, BL], bf, kind="Internal")
    cc2_d = nc.dram_tensor("cc2", [NCORE, 128, 14, BL], bf,
                           kind="Internal", addr_space="Shared")
    ag3_d = nc.dram_tensor("ag3", [128, 22, BL], bf, kind="Internal")
    cc3_d = nc.dram_tensor("cc3", [NCORE, 128, 22, BL], bf,
                           kind="Internal", addr_space="Shared")
    rs_in = nc.dram_tensor("rs_in", [128, 256], f32, kind="Internal")
    rs_out = nc.dram_tensor("rs_out", [BL, 256], f32, kind="Internal")
    RG = [list(range(NCORE))]

    from contextlib import ExitStack
    with tile.TileContext(nc) as tc, ExitStack() as ctx:
        wp = ctx.enter_context(tc.tile_pool(name="weights", bufs=1))
        xp_pool = ctx.enter_context(tc.tile_pool(name="xblk", bufs=2))
        blkp = ctx.enter_context(tc.tile_pool(name="blk", bufs=3))
        psp = ctx.enter_context(tc.tile_pool(name="psum", bufs=2, space="PSUM"))
        psq = ctx.enter_context(tc.tile_pool(name="psum2", bufs=1, space="PSUM"))
        vp = ctx.enter_context(tc.tile_pool(name="vtmp", bufs=4))
        pers = ctx.enter_context(tc.tile_pool(name="persist", bufs=1))

        w1t = wp.tile([96, 5, 3, 128], bf)
        w2t = wp.tile([96, 5, 2, 128], bf)
        w3t = wp.tile([110, 5, 3, 128], bf)
        b1t = wp.tile([128, 1], f32)
        b2t = wp.tile([128, 1], f32)
        b3t = wp.tile([128, 1], f32)
        for dst, src, q in ((w1t, w1_d, nc.sync), (b1t, b1_d, nc.sync),
                            (b2t, b2_d, nc.scalar), (b3t, b3_d, nc.scalar)):
            q.dma_start(dst[:], src[:])

        pool1 = pers.tile([128, 60, 4, 60], bf)
        pool2 = pers.tile([128, 32, 8, 32], bf)
        feat_sb = pers.tile([128, 49, BL], f32)   # s-major: s = 7*y + x
        fz = pers.tile([128, 50, BL], bf)         # bf16 AG payload, row 49 = logits
        # only the pad borders are ever read as zeros; memset them on DVE so
        # the gpsimd DMA queue isn't blocked at kernel start
        for t, n in ((pool1, 60), (pool2, 32)):
            nc.vector.memset(t[:, 0:2, :, :], 0.0)
            nc.vector.memset(t[:, n - 2:n, :, :], 0.0)
            nc.vector.memset(t[:, :, :, 0:2], 0.0)
            nc.vector.memset(t[:, :, :, n - 2:n], 0.0)

        # ---------------- conv stack (identical to build_conv_module)
        for b in range(28):
            xt = xp_pool.tile([96, 3, BL, 116], bf)
            qj = (nc.sync, nc.scalar, nc.gpsimd) if b == 0 else \
                 (nc.sync, nc.sync, nc.gpsimd)
            for j, q in zip(range(3), qj):
                q.dma_start(
                    xt[:, j].rearrange("p i x -> p (i x)"),
                    xblk_d[12 * j:12 * j + 12, 4 * b:4 * b + 8]
                    .rearrange("c y i x -> c y (i x)"))
            if b == 0:
                # w2t/w3t ride after the first input block so the DMA engines
                # serve L1's critical path first
                nc.scalar.dma_start(w2t[:], w2_d[:])
                nc.scalar.dma_start(w3t[:], w3_d[:])
            for sg in range(4):
                ps = psp.tile([128, 4, 112], mybir.dt.float32)
                k = 0
                for kx in range(5):
                    for j in range(3):
                        nc.tensor.matmul(
                            ps[:], w1t[:, kx, j, :],
                            xt[:, j, 4 * sg:4 * sg + 4, kx:kx + 112],
                            start=(k == 0), stop=(k == 14))
                        k += 1
                ev = vp.tile([128, 4, 112], bf)
                nc.scalar.activation(out=ev[:], in_=ps[:], func=RELU,
                                     bias=b1t[:], scale=1.0)
                xpo = vp.tile([32, 4, 4, 56], bf)
                for yo in range(4):
                    nc.vector.tensor_tensor(
                        xpo[:, yo], ev[32 * yo:32 * yo + 32, :, 0::2],
                        ev[32 * yo:32 * yo + 32, :, 1::2], MAX)
                for yy in range(2):
                    nc.vector.tensor_tensor(
                        pool1[32 * sg:32 * sg + 32, 2 + 2 * b + yy, :, 2:58],
                        xpo[:, 2 * yy], xpo[:, 2 * yy + 1], MAX)

        # expert L1 weights: 8 half-loads pinned to successive L2 blocks via
        # a dummy WAW dep (tiny copy from a pool2 element written the block
        # before) so the scheduler cannot hoist the 2.3us transfers into L1
        ew1m = []
        for m in range(4):
            w = wp.tile([128, 49, 128], bf, name=f"ew1m{m}")
            ew1m.append(w)
        EW1_Q = [0, 13, 25, 37, 49]
        EW1_BLOCKS = (4, 5, 7, 8, 10, 11, 13, 14,
                      16, 17, 19, 20, 22, 23, 25, 26)
        EW1_SCHED = {bb: (i // 4, i % 4) for i, bb in enumerate(EW1_BLOCKS)}
        L2_QUEUES = (nc.sync, nc.sync, nc.sync, nc.scalar,
                     nc.scalar, nc.gpsimd, nc.gpsimd, nc.gpsimd)
        for b in range(28):
            bt = blkp.tile([96, 2, 4, 4, 60], bf)
            for j in range(2):
                for q in range(4):
                    src = pool1[32 * q + 16 * j:32 * q + 16 * j + 16,
                                2 * b:2 * b + 6, :, :]
                    L2_QUEUES[4 * j + q].dma_start(
                        bt[:, j, q].rearrange("p i x -> p (i x)"),
                        src.rearrange("c y i x -> c y (i x)"))
            for sg in range(2):
                ps = psp.tile([128, 2, 4, 56], mybir.dt.float32)
                k = 0
                for kx in range(5):
                    for j in range(2):
                        nc.tensor.matmul(
                            ps[:], w2t[:, kx, j, :],
                            bt[:, j, 2 * sg:2 * sg + 2, :, kx:kx + 56],
                            start=(k == 0), stop=(k == 9))
                        k += 1
                ev = vp.tile([128, 2, 4, 56], bf, tag="ev2")
                nc.scalar.activation(out=ev[:], in_=ps[:], func=RELU,
                                     bias=b2t[:], scale=1.0)
                xpo = vp.tile([64, 2, 2, 4, 28], bf, tag="xpo2")
                for yo in range(2):
                    nc.vector.tensor_tensor(
                        xpo[:, yo], ev[64 * yo:64 * yo + 64, :, :, 0::2],
                        ev[64 * yo:64 * yo + 64, :, :, 1::2], MAX)
                nc.vector.tensor_tensor(
                    pool2[64 * sg:64 * sg + 64, 2 + b, :, 2:30]
                    .rearrange("c (q i) x -> c q i x", q=2),
                    xpo[:, 0], xpo[:, 1], MAX)
            if b in EW1_SCHED:
                m, hh = EW1_SCHED[b]
                s0, s1 = EW1_Q[hh], EW1_Q[hh + 1]
                nc.vector.tensor_copy(
                    ew1m[m][0:1, s0:s0 + 1, 0:1].rearrange("c s h -> c (s h)"),
                    pool2[0:1, 1 + b:2 + b, 0:1, 2:3]
                    .rearrange("c y i x -> c (y i x)"))
                nc.sync.dma_start(ew1m[m][:, s0:s1, :], ew1_d[m, :, s0:s1, :])

        # router weights + payload setup; logits accumulate into ps_r as
        # feat rows complete (interleaved two blocks late so PE never waits)
        rwt = wp.tile([128, 49, 8], f32)
        rbt = wp.tile([1, 8], f32)
        ones32 = wp.tile([1, BL], f32)
        ones = wp.tile([1, 128], bf)
        oht = wp.tile([128, 8], f32)
        nc.scalar.dma_start(rwt[:], rwt_d[:])
        nc.scalar.dma_start(rbt[:], rb_d[:])
        nc.scalar.dma_start(oht[:], oh_d[:])
        nc.vector.memset(ones[:], 1.0)
        nc.vector.memset(ones32[:], 1.0)
        nc.vector.memset(fz[:, 49, :], 0.0)
        ps_r = psq.tile([BL, 8], mybir.dt.float32, tag="psr")

        L3_QUEUES = (nc.sync, nc.gpsimd, nc.sync, nc.gpsimd, nc.sync, nc.scalar)
        L3_QUEUES_NOPOOL = (nc.sync, nc.scalar, nc.sync, nc.scalar,
                            nc.sync, nc.scalar)
        for b in range(28):
            # blocks right after an in-loop collective: keep Pool free so the
            # collective's sem wait can't stall the gather feed
            qs = L3_QUEUES_NOPOOL if b in (8, 9, 16, 17) else L3_QUEUES
            bt = blkp.tile([110, 3, 2, 8, 32], bf, tag="bt3")
            for j, (coff, ccnt) in enumerate(L3_CH):
                for h in range(2):
                    src = pool2[64 * h + coff:64 * h + coff + ccnt, b:b + 5, :, :]
                    qs[2 * j + h].dma_start(
                        bt[0:ccnt * 5, j, h].rearrange("p i x -> p (i x)"),
                        src.rearrange("c y i x -> c y (i x)"))
            ps = psp.tile([128, 2, 8, 28], mybir.dt.float32)
            k = 0
            for kx in range(5):
                for j, (coff, ccnt) in enumerate(L3_CH):
                    nc.tensor.matmul(
                        ps[:], w3t[0:ccnt * 5, kx, j, :],
                        bt[0:ccnt * 5, j, :, :, kx:kx + 28],
                        start=(k == 0), stop=(k == 14))
                    k += 1
            ev3 = vp.tile([128, BL, 28], mybir.dt.float32, tag="ev3", bufs=6)
            nc.scalar.activation(
                out=ev3[:], in_=ps[:].rearrange("c h i x -> c (h i) x"),
                func=RELU, bias=b3t[:], scale=1.0)
            # streamed maxpool 28->14 (x now, y across block pairs) and
            # avgpool 14->7 (x now, y across row pairs)
            xr = vp.tile([128, BL, 14], mybir.dt.float32,
                         tag=f"xr{b % 2}", bufs=3, name=f"xr_{b}")
            nc.vector.tensor_tensor(xr[:], ev3[:, :, 0::2], ev3[:, :, 1::2], MAX)
            if b % 2 == 0:
                xr_prev = xr
            else:
                yp = (b - 1) // 2
                mrow = vp.tile([128, BL, 14], mybir.dt.float32, tag="mrow",
                               bufs=3, name=f"mrow_{b}")
                nc.vector.tensor_tensor(mrow[:], xr_prev[:], xr[:], MAX)
                avx = vp.tile([128, BL, 7], mybir.dt.float32,
                              tag=f"av{yp % 2}", bufs=3, name=f"avx_{b}")
                nc.vector.tensor_tensor(avx[:], mrow[:, :, 0::2],
                                        mrow[:, :, 1::2], ADD)
                if yp % 2 == 0:
                    avx_prev = avx
                else:
                    # feat rows stored as 4*avgpool (0.25 folded into rwt/ew1
                    # host-side); fz write is independent of feat_sb so the
                    # AG payload chain is 1 op shorter
                    yr = (yp - 1) // 2
                    nc.vector.tensor_tensor(
                        fz[:, 7 * yr:7 * yr + 7, :]
                        .rearrange("c s t -> c t s"),
                        avx_prev[:], avx[:], ADD)
                    nc.vector.tensor_tensor(
                        feat_sb[:, 7 * yr:7 * yr + 7, :]
                        .rearrange("c s t -> c t s"),
                        avx_prev[:], avx[:], ADD)
            if b >= 5 and (b - 5) % 4 == 0:
                yrr = (b - 5) // 4
                for s in range(7 * yrr, 7 * yrr + 7):
                    nc.tensor.matmul(ps_r[:], feat_sb[:, s, :], rwt[:, s, :],
                                     start=(s == 0), stop=False)
            if b == 7:
                # feat rows 0,1 (s 0..14) are final; AllGather chunk 1
                nc.gpsimd.dma_start(ag1_d[:], fz[:, 0:14, :])
                nc.gpsimd.collective_compute(
                    "AllGather", mybir.AluOpType.bypass, replica_groups=RG,
                    ins=[ag1_d[:]], outs=[cc1_d[:]])
            if b == 15:
                # feat rows 2,3 (s 14..28): AllGather chunk 2
                nc.gpsimd.dma_start(ag2_d[:], fz[:, 14:28, :])
                nc.gpsimd.collective_compute(
                    "AllGather", mybir.AluOpType.bypass, replica_groups=RG,
                    ins=[ag2_d[:]], outs=[cc2_d[:]])

        # ---------------- router tail + final AG chunk FIRST (critical path:
        # logits -> ag3 -> collective3), then staging/loads that overlap it
        for s in range(42, 49):
            nc.tensor.matmul(ps_r[:], feat_sb[:, s, :], rwt[:, s, :],
                             start=False, stop=False)
        nc.tensor.matmul(ps_r[:], ones32[:], rbt[:], start=False, stop=True)
        lgl = vp.tile([BL, 8], f32, tag="lgl", bufs=1)
        nc.vector.tensor_copy(lgl[:], ps_r[:])
        nc.vector.tensor_copy(fz[0:BL, 49, :], lgl[:].bitcast(bf))
        nc.scalar.dma_start(ag3_d[:], fz[:, 28:50, :])
        nc.gpsimd.collective_compute(
            "AllGather", mybir.AluOpType.bypass, replica_groups=RG,
            ins=[ag3_d[:]], outs=[cc3_d[:]])

        featT = pers.tile([128, NCORE, 49, BL], bf, tag="pool2")
        nc.sync.dma_start(featT[:, :, 0:14, :],
                          cc1_d[:].rearrange("r c s t -> c r s t"))
        nc.sync.dma_start(featT[:, :, 14:28, :],
                          cc2_d[:].rearrange("r c s t -> c r s t"))
        ew2t = wp.tile([128, 4, 256], bf)
        eb1t = wp.tile([128, 4], f32)
        eb2t = wp.tile([1, 256], bf)
        nc.sync.dma_start(ew2t[:], ew2_d[:])
        nc.scalar.dma_start(eb1t[:], eb1_d[:])
        nc.scalar.dma_start(eb2t[:], eb2r_d[:])
        # logits first so the softmax overlaps the chunk-3 feat staging
        lg = vp.tile([128, 8], f32, tag="lg")
        for r in range(NCORE):
            nc.sync.dma_start(
                lg[BL * r:BL * r + BL, :].bitcast(bf), cc3_d[r, 0:BL, 21, :])
        nc.scalar.dma_start(
            featT[:, :, 28:49, :],
            cc3_d[:, :, 0:21, :].rearrange("r c s t -> c r s t"))
        m1 = vp.tile([128, 1], f32, tag="m1")
        nc.vector.tensor_reduce(m1[:], lg[:], axis=mybir.AxisListType.X, op=MAX)
        negm1 = vp.tile([128, 1], f32, tag="negm1")
        nc.vector.tensor_scalar_mul(negm1[:], m1[:], -1.0)
        eqm = vp.tile([128, 8], f32, tag="eqm")
        nc.vector.tensor_scalar(eqm[:], lg[:], m1[:], None,
                                op0=mybir.AluOpType.is_equal)
        pen = vp.tile([128, 8], f32, tag="pen")
        nc.vector.tensor_scalar_mul(pen[:], eqm[:], -1e30)
        msk = vp.tile([128, 8], f32, tag="msk")
        nc.vector.tensor_tensor(msk[:], lg[:], pen[:], ADD)
        m2 = vp.tile([128, 1], f32, tag="m2")
        nc.vector.tensor_reduce(m2[:], msk[:], axis=mybir.AxisListType.X, op=MAX)
        z = vp.tile([128, 8], f32, tag="z")
        nc.scalar.activation(out=z[:], in_=lg[:],
                             func=mybir.ActivationFunctionType.Exp,
                             bias=negm1[:], scale=1.0)
        e2 = vp.tile([128, 1], f32, tag="e2")
        nc.scalar.activation(out=e2[:], in_=m2[:],
                             func=mybir.ActivationFunctionType.Exp,
                             bias=negm1[:], scale=1.0)
        den = vp.tile([128, 1], f32, tag="den")
        nc.vector.tensor_scalar_add(den[:], e2[:], 1.0)
        rden = vp.tile([128, 1], f32, tag="rden")
        nc.vector.reciprocal(rden[:], den[:])
        g = vp.tile([128, 8], f32, tag="g")
        nc.vector.tensor_scalar(g[:], lg[:], m2[:], None,
                                op0=mybir.AluOpType.is_ge)
        zg = vp.tile([128, 8], f32, tag="zg")
        nc.vector.tensor_tensor(zg[:], z[:], g[:], mybir.AluOpType.mult)
        coef = vp.tile([128, 8], f32, tag="coef")
        nc.vector.tensor_scalar_mul(coef[:], zg[:], rden[:])
        cm = vp.tile([128, 8], f32, tag="cm")
        nc.vector.tensor_tensor(cm[:], coef[:], oht[:], mybir.AluOpType.mult)
        coefe = vp.tile([128, 1], f32, tag="coefe")
        nc.vector.tensor_reduce(coefe[:], cm[:], axis=mybir.AxisListType.X, op=ADD)

        # ---------------- expert MLP: s-outer so chunk-1 rows (s<28) start
        # before chunk 2 lands; 4 live PSUM accumulators
        psm = [psq.tile([128, 128], mybir.dt.float32, tag=f"psm{m}",
                        name=f"psm{m}")
               for m in range(4)]
        for s in range(49):
            for m in range(4):
                nc.tensor.matmul(psm[m][:], ew1m[m][:, s, :], featT[:, :, s, :],
                                 start=(s == 0), stop=(s == 48))
        hid = pers.tile([128, 4, 128], bf)
        for m in range(4):
            nc.scalar.activation(out=hid[:, m, :], in_=psm[m][:], func=RELU,
                                 bias=eb1t[:, m:m + 1], scale=1.0)
        ps2 = psq.tile([128, 256], mybir.dt.float32, tag="ps2")
        for m in range(4):
            nc.tensor.matmul(ps2[:], hid[:, m, :], ew2t[:, m, :],
                             start=(m == 0), stop=False)
        nc.tensor.matmul(ps2[:], ones[:], eb2t[:], start=False, stop=True)
        wout = vp.tile([128, 256], f32, tag="wout", bufs=1)
        nc.vector.tensor_scalar_mul(wout[:], ps2[:], coefe[:])
        nc.sync.dma_start(rs_in[:], wout[:])

        # ---------------- ReduceScatter (sum over experts, keep token shard)
        nc.gpsimd.collective_compute(
            "ReduceScatter", mybir.AluOpType.add, replica_groups=RG,
            ins=[rs_in[:]], outs=[rs_out[:]])

        # ---------------- L2 normalize token shard
        nsb = vp.tile([BL, 256], f32, tag="nsb", bufs=1)
        nc.sync.dma_start(nsb[:], rs_out[:])
        sq = vp.tile([BL, 256], f32, tag="sq", bufs=1)
        nc.vector.tensor_tensor(sq[:], nsb[:], nsb[:], mybir.AluOpType.mult)
        ss = vp.tile([BL, 1], f32, tag="ss")
        nc.vector.tensor_reduce(ss[:], sq[:], axis=mybir.AxisListType.X, op=ADD)
        nrm = vp.tile([BL, 1], f32, tag="nrm")
        nc.scalar.activation(out=nrm[:], in_=ss[:],
                             func=mybir.ActivationFunctionType.Sqrt, scale=1.0)
        nc.vector.tensor_scalar_max(nrm[:], nrm[:], 1e-12)
        rn = vp.tile([BL, 1], f32, tag="rn")
        nc.vector.reciprocal(rn[:], nrm[:])
        yt = vp.tile([BL, 256], f32, tag="yt", bufs=1)
        nc.vector.tensor_scalar_mul(yt[:], nsb[:], rn[:])
        nc.sync.dma_start(y_d[:], yt[:])
    nc.compile()
    return nc


def kernel_fused(inputs):
    conv_keys = ("x", "conv1_w", "conv1_b", "bn1_g", "bn1_b", "bn1_m", "bn1_v",
                 "conv2_w", "conv2_b", "bn2_g", "bn2_b", "bn2_m", "bn2_v",
                 "conv3_w", "conv3_b", "bn3_g", "bn3_b", "bn3_m", "bn3_v")
    in_maps = _prep_conv_inputs(*[inputs[k] for k in conv_keys])
    rw = inputs["router_w"]
    shared = {
        "rwt": np.ascontiguousarray(
            (0.25 * rw.T).reshape(128, 49, 8).astype(np.float32)),
        "rb": inputs["router_b"].astype(np.float32)[None, :],
    }
    for e in range(NCORE):
        oh = np.zeros((128, 8), np.float32)
        oh[:, e] = 1.0
        in_maps[e].update(shared)
        in_maps[e]["ew1"] = np.ascontiguousarray(
            (0.25 * inputs["ew1"][e]).reshape(128, 49, 4, 128)
            .transpose(2, 0, 1, 3).astype(BF16))
        in_maps[e]["ew2"] = np.ascontiguousarray(
            inputs["ew2"][e].reshape(4, 128, 256).transpose(1, 0, 2).astype(BF16))
        in_maps[e]["eb1"] = np.ascontiguousarray(
            inputs["eb1"][e].reshape(4, 128).T.astype(np.float32))
        in_maps[e]["eb2r"] = inputs["eb2"][e].astype(BF16)[None, :]
        in_maps[e]["oh"] = oh
    if "fused_nc" not in _CACHE:
        _CACHE["fused_nc"] = build_fused_module()
    res = run_bass_kernel_spmd(_CACHE["fused_nc"], in_maps,
                               core_ids=list(range(NCORE)))
    LAST["a"] = res
    LAST.pop("b", None)
    return np.concatenate([res.results[c]["y"] for c in range(NCORE)], axis=0)



# revision 44
# speedup vs baseline: 1.0019x; 1.0019x over previous
"""Trainium2 Bass kernel for nn_Backbone_3143916060887 (moe_routing).

Pipeline: 3x (5x5 conv + folded-BN + ReLU + maxpool2) -> avgpool2 -> feat
          -> router top-2 -> expert MLPs -> weighted combine -> L2 normalize.

Single fused SPMD launch on 8 cores: convs are data-parallel over batch
(16 images/core); the expert phase is expert-parallel (1 expert/core, all
128 tokens). feat + on-chip f32 router logits are AllGathered in THREE
chunks (feat y-row groups at L3 blocks b=7, b=15, and the tail) so most of
the collective latency hides under L3 compute; expert partials are summed
with a ReduceScatter back to the token-owner core, which L2-normalizes.

Conv strategy: tap-accumulation matmuls with output rows packed into PSUM
partitions, M=(y_off, Cout), K=(Cin, y_rel), kx as free-dim shifts. Maxpool
runs on DVE with cross-partition-base tensor_tensor max ops; BN bias + ReLU
are applied once per pooled value by ScalarE (bias commutes with max). The
avgpool 0.25 scale is folded into the router/expert-1 weights host-side.
Block-gather DMAs are spread across the SP/Act/Pool queues (the shared HWDGE
device otherwise gates the PE), and expert weights stream in pinned ~1.2us
quarter-loads during L2 via dummy WAW deps so the list scheduler cannot
hoist them into L1's input feed.

Fallback: if the fused build fails, a two-launch path (conv kernel + expert
kernel with host routing between) produces identical results.
"""

import numpy as np
import ml_dtypes

try:  # persistent XLA/NEFF cache so repeat processes skip the ~60s compile
    import jax
    jax.config.update("jax_compilation_cache_dir", "/tmp/jax_cache")
    jax.config.update("jax_persistent_cache_min_entry_size_bytes", -1)
    jax.config.update("jax_persistent_cache_min_compile_time_secs", 0.0)
except Exception:
    pass

import concourse.bass as bass
import concourse.bacc as bacc
import concourse.mybir as mybir
import concourse.tile as tile
from concourse.bass_utils import run_bass_kernel_spmd

NCORE = 8
B, CIN, H, W = 128, 36, 112, 112
BL = B // NCORE                      # 16 images per core
E, F, HID, D = 8, 7 * 7 * 128, 512, 256
BN_EPS = 1e-5
BF16 = ml_dtypes.bfloat16
RELU = mybir.ActivationFunctionType.Relu
MAX = mybir.AluOpType.max
ADD = mybir.AluOpType.add

# L3 K-chunking: (c, y_rel in 0..4), chunks of 22/22/20 input channels
L3_CH = [(0, 22), (22, 22), (44, 20)]


# ---------------------------------------------------------------------------
# host-side weight preparation
# ---------------------------------------------------------------------------

def _fold_bn(w, b, g, beta, m, v):
    s = g.astype(np.float64) / np.sqrt(v.astype(np.float64) + BN_EPS)
    return (w.astype(np.float64) * s[:, None, None, None],
            (b.astype(np.float64) - m.astype(np.float64)) * s + beta.astype(np.float64))


def _l1_weights(w):  # w float64 [32, 36, 5, 5] -> [96, 5, 3, 128] bf16
    out = np.zeros((96, 5, 3, 128), np.float64)
    for kx in range(5):
        for j in range(3):
            for c_r in range(12):
                for y_rel in range(8):
                    p = c_r * 8 + y_rel
                    for y_off in range(4):
                        ky = y_rel - y_off
                        if 0 <= ky < 5:
                            out[p, kx, j, y_off * 32:(y_off + 1) * 32] = \
                                w[:, j * 12 + c_r, ky, kx]
    return out.astype(BF16)


def _l2_weights(w):  # w float64 [64, 32, 5, 5] -> [96, 5, 2, 128] bf16
    out = np.zeros((96, 5, 2, 128), np.float64)
    for kx in range(5):
        for j in range(2):
            for c_r in range(16):
                for y_rel in range(6):
                    p = c_r * 6 + y_rel
                    for y_off in range(2):
                        ky = y_rel - y_off
                        if 0 <= ky < 5:
                            out[p, kx, j, y_off * 64:(y_off + 1) * 64] = \
                                w[:, j * 16 + c_r, ky, kx]
    return out.astype(BF16)


def _l3_weights(w):  # w float64 [128, 64, 5, 5] -> [110, 5, 3, 128] bf16
    out = np.zeros((110, 5, 3, 128), np.float64)
    for kx in range(5):
        for j, (coff, ccnt) in enumerate(L3_CH):
            for c_r in range(ccnt):
                for y_rel in range(5):
                    out[c_r * 5 + y_rel, kx, j, :] = w[:, coff + c_r, y_rel, kx]
    return out.astype(BF16)


def _l1_blocks(xc):
    """xc [16, 36, 112, 112] f32 -> [36, 116, 16, 116] bf16 padded (c, y, i, x).

    The (c_r, y_rel) block replication happens in the per-block DMA read
    pattern on-chip, so the host only ships the padded tensor once."""
    xpad = np.zeros((BL, 36, 116, 116), BF16)
    xpad[:, :, 2:114, 2:114] = xc.astype(BF16)
    return np.ascontiguousarray(xpad.transpose(1, 2, 0, 3))


# ---------------------------------------------------------------------------
# kernel A: conv stack -> feat  (data-parallel, 16 images/core)
# ---------------------------------------------------------------------------

def build_conv_module():
    nc = bacc.Bacc("TRN2", target_bir_lowering=False, debug=False,
                   num_devices=NCORE)
    f32, bf = mybir.dt.float32, mybir.dt.bfloat16
    xblk_d = nc.dram_tensor("xblk", [36, 116, BL, 116], bf, kind="ExternalInput")
    w1_d = nc.dram_tensor("w1t", [96, 5, 3, 128], bf, kind="ExternalInput")
    w2_d = nc.dram_tensor("w2t", [96, 5, 2, 128], bf, kind="ExternalInput")
    w3_d = nc.dram_tensor("w3t", [110, 5, 3, 128], bf, kind="ExternalInput")
    b1_d = nc.dram_tensor("b1", [128, 1], f32, kind="ExternalInput")
    b2_d = nc.dram_tensor("b2", [128, 1], f32, kind="ExternalInput")
    b3_d = nc.dram_tensor("b3", [128, 1], f32, kind="ExternalInput")
    feat_d = nc.dram_tensor("feat", [128, BL, 49], f32, kind="ExternalOutput")

    from contextlib import ExitStack
    with tile.TileContext(nc) as tc, ExitStack() as ctx:
        wp = ctx.enter_context(tc.tile_pool(name="weights", bufs=1))
        xp_pool = ctx.enter_context(tc.tile_pool(name="xblk", bufs=2))
        blkp = ctx.enter_context(tc.tile_pool(name="blk", bufs=2))
        psp = ctx.enter_context(tc.tile_pool(name="psum", bufs=4, space="PSUM"))
        vp = ctx.enter_context(tc.tile_pool(name="vtmp", bufs=4))
        pers = ctx.enter_context(tc.tile_pool(name="persist", bufs=1))

        w1t = wp.tile([96, 5, 3, 128], bf)
        w2t = wp.tile([96, 5, 2, 128], bf)
        w3t = wp.tile([110, 5, 3, 128], bf)
        b1t = wp.tile([128, 1], f32)
        b2t = wp.tile([128, 1], f32)
        b3t = wp.tile([128, 1], f32)
        for dst, src in ((w1t, w1_d), (w2t, w2_d), (w3t, w3_d),
                         (b1t, b1_d), (b2t, b2_d), (b3t, b3_d)):
            nc.sync.dma_start(dst[:], src[:])

        # persistent activations: pool1 [(q4,c32), (60 y, 4 i, 60 x)], img = q*4+i
        pool1 = pers.tile([128, 60, 4, 60], bf)
        # pool2 [(h2,c64), (32 y, 8 i, 32 x)], img = h*8+i
        pool2 = pers.tile([128, 32, 8, 32], bf)
        conv3 = pers.tile([128, BL, 28, 28], f32)
        feat_sb = pers.tile([128, BL, 7, 7], f32)
        nc.gpsimd.memset(pool1[:], 0.0)
        nc.gpsimd.memset(pool2[:], 0.0)

        # ---------------- layer 1: 36 -> 32, 112x112 -> pool 56x56
        for b in range(28):
            xt = xp_pool.tile([96, 3, BL, 116], bf)
            for j in range(3):
                nc.sync.dma_start(
                    xt[:, j].rearrange("p i x -> p (i x)"),
                    xblk_d[12 * j:12 * j + 12, 4 * b:4 * b + 8]
                    .rearrange("c y i x -> c y (i x)"))
            for sg in range(4):                      # 4 images per matmul group
                ps = psp.tile([128, 4, 112], mybir.dt.float32)
                k = 0
                for kx in range(5):
                    for j in range(3):
                        nc.tensor.matmul(
                            ps[:], w1t[:, kx, j, :],
                            xt[:, j, 4 * sg:4 * sg + 4, kx:kx + 112],
                            start=(k == 0), stop=(k == 14))
                        k += 1
                ev = vp.tile([128, 4, 112], bf)
                nc.scalar.activation(out=ev[:], in_=ps[:], func=RELU,
                                     bias=b1t[:], scale=1.0)
                # x-pool per y_off slice into base-0 tile (legal SB+SB TT
                # needs equal input bases; base-remap happens on the write)
                xpo = vp.tile([32, 4, 4, 56], bf)
                for yo in range(4):
                    nc.vector.tensor_tensor(
                        xpo[:, yo], ev[32 * yo:32 * yo + 32, :, 0::2],
                        ev[32 * yo:32 * yo + 32, :, 1::2], MAX)
                for yy in range(2):                  # pooled rows 2b, 2b+1
                    nc.vector.tensor_tensor(
                        pool1[32 * sg:32 * sg + 32, 2 + 2 * b + yy, :, 2:58],
                        xpo[:, 2 * yy], xpo[:, 2 * yy + 1], MAX)

        # ---------------- layer 2: 32 -> 64, 56x56 -> pool 28x28
        EW1_SCHED = {}
        ew1m, ew1_d = [], None
        for b in range(28):
            bt = blkp.tile([96, 2, 4, 4, 60], bf)  # p, j, q, i, x
            for j in range(2):
                for q in range(4):
                    src = pool1[32 * q + 16 * j:32 * q + 16 * j + 16,
                                2 * b:2 * b + 6, :, :]
                    nc.sync.dma_start(
                        bt[:, j, q].rearrange("p i x -> p (i x)"),
                        src.rearrange("c y i x -> c y (i x)"))
            for sg in range(2):                      # 8 images per matmul group
                ps = psp.tile([128, 2, 4, 56], mybir.dt.float32)
                k = 0
                for kx in range(5):
                    for j in range(2):
                        nc.tensor.matmul(
                            ps[:], w2t[:, kx, j, :],
                            bt[:, j, 2 * sg:2 * sg + 2, :, kx:kx + 56],
                            start=(k == 0), stop=(k == 9))
                        k += 1
                ev = vp.tile([128, 2, 4, 56], bf, tag="ev2")
                nc.scalar.activation(out=ev[:], in_=ps[:], func=RELU,
                                     bias=b2t[:], scale=1.0)
                xpo = vp.tile([64, 2, 2, 4, 28], bf, tag="xpo2")
                for yo in range(2):
                    nc.vector.tensor_tensor(
                        xpo[:, yo], ev[64 * yo:64 * yo + 64, :, :, 0::2],
                        ev[64 * yo:64 * yo + 64, :, :, 1::2], MAX)
                nc.vector.tensor_tensor(
                    pool2[64 * sg:64 * sg + 64, 2 + b, :, 2:30]
                    .rearrange("c (q i) x -> c q i x", q=2),
                    xpo[:, 0], xpo[:, 1], MAX)
            if b in EW1_SCHED:
                m, hh = EW1_SCHED[b]
                s0, s1 = EW1_Q[hh], EW1_Q[hh + 1]
                nc.vector.tensor_copy(
                    ew1m[m][0:1, s0:s0 + 1, 0:1].rearrange("c s h -> c (s h)"),
                    pool2[0:1, 1 + b:2 + b, 0:1, 2:3]
                    .rearrange("c y i x -> c (y i x)"))
                nc.sync.dma_start(ew1m[m][:, s0:s1, :], ew1_d[m, :, s0:s1, :])

        # ---------------- layer 3: 64 -> 128, 28x28 (pool later)
        for b in range(28):
            bt = blkp.tile([110, 3, 2, 8, 32], bf, tag="bt3")  # p, j, h, i, x
            for j, (coff, ccnt) in enumerate(L3_CH):
                for h in range(2):
                    src = pool2[64 * h + coff:64 * h + coff + ccnt, b:b + 5, :, :]
                    nc.sync.dma_start(
                        bt[0:ccnt * 5, j, h].rearrange("p i x -> p (i x)"),
                        src.rearrange("c y i x -> c y (i x)"))
            ps = psp.tile([128, 2, 8, 28], mybir.dt.float32)
            k = 0
            for kx in range(5):
                for j, (coff, ccnt) in enumerate(L3_CH):
                    nc.tensor.matmul(
                        ps[:], w3t[0:ccnt * 5, kx, j, :],
                        bt[0:ccnt * 5, j, :, :, kx:kx + 28],
                        start=(k == 0), stop=(k == 14))
                    k += 1
            nc.scalar.activation(
                out=conv3[:, :, b, :],
                in_=ps[:].rearrange("c h i x -> c (h i) x"),
                func=RELU, bias=b3t[:], scale=1.0)

        # ---------------- maxpool 28->14, avgpool 14->7 (per 4-image chunk)
        for i4 in range(4):
            c4 = conv3[:, 4 * i4:4 * i4 + 4]
            mx = vp.tile([128, 4, 28, 14], mybir.dt.float32, tag="mx")
            nc.vector.tensor_tensor(mx[:], c4[:, :, :, 0::2], c4[:, :, :, 1::2], MAX)
            mxy = vp.tile([128, 4, 14, 14], mybir.dt.float32, tag="mxy")
            nc.vector.tensor_tensor(mxy[:], mx[:, :, 0::2, :], mx[:, :, 1::2, :], MAX)
            ax = vp.tile([128, 4, 14, 7], mybir.dt.float32, tag="ax")
            nc.vector.tensor_tensor(ax[:], mxy[:, :, :, 0::2], mxy[:, :, :, 1::2], ADD)
            ay = vp.tile([128, 4, 7, 7], mybir.dt.float32, tag="ay")
            nc.vector.tensor_tensor(ay[:], ax[:, :, 0::2, :], ax[:, :, 1::2, :], ADD)
            nc.vector.tensor_scalar_mul(feat_sb[:, 4 * i4:4 * i4 + 4], ay[:], 0.25)

        nc.sync.dma_start(feat_d[:].rearrange("c i s -> c (i s)"),
                          feat_sb[:].rearrange("c i y x -> c (i y x)"))
    nc.compile()
    return nc


# ---------------------------------------------------------------------------
# kernel B: one expert for all 128 tokens  (expert-parallel)
# ---------------------------------------------------------------------------

def build_expert_module():
    nc = bacc.Bacc("TRN2", target_bir_lowering=False, debug=False,
                   num_devices=NCORE)
    f32, bf = mybir.dt.float32, mybir.dt.bfloat16
    ft_d = nc.dram_tensor("featT", [128, 49, 128], bf, kind="ExternalInput")
    ew1_d = nc.dram_tensor("ew1", [128, 49, 512], bf, kind="ExternalInput")
    ew2_d = nc.dram_tensor("ew2", [128, 4, 256], bf, kind="ExternalInput")
    eb1_d = nc.dram_tensor("eb1", [128, 4], f32, kind="ExternalInput")
    coef_d = nc.dram_tensor("coef", [128, 1], f32, kind="ExternalInput")
    out_d = nc.dram_tensor("partial", [128, 256], f32, kind="ExternalOutput")

    from contextlib import ExitStack
    with tile.TileContext(nc) as tc, ExitStack() as ctx:
        wp = ctx.enter_context(tc.tile_pool(name="w", bufs=1))
        psp = ctx.enter_context(tc.tile_pool(name="psum", bufs=4, space="PSUM"))
        ft = wp.tile([128, 49, 128], bf)
        ew1 = wp.tile([128, 49, 512], bf)
        ew2 = wp.tile([128, 4, 256], bf)
        eb1 = wp.tile([128, 4], f32)
        coef = wp.tile([128, 1], f32)
        hid = wp.tile([128, 4, 128], bf)
        outb = wp.tile([128, 256], f32)
        nc.sync.dma_start(ft[:], ft_d[:])
        nc.sync.dma_start(ew1[:], ew1_d[:])
        nc.sync.dma_start(ew2[:], ew2_d[:])
        nc.sync.dma_start(eb1[:], eb1_d[:])
        nc.sync.dma_start(coef[:], coef_d[:])
        for m in range(4):
            ps = psp.tile([128, 128], mybir.dt.float32)
            for s in range(49):
                nc.tensor.matmul(ps[:], ew1[:, s, 128 * m:128 * m + 128],
                                 ft[:, s, :], start=(s == 0), stop=(s == 48))
            nc.scalar.activation(out=hid[:, m, :], in_=ps[:], func=RELU,
                                 bias=eb1[:, m:m + 1], scale=1.0)
        ps2 = psp.tile([128, 256], mybir.dt.float32)
        for m in range(4):
            nc.tensor.matmul(ps2[:], hid[:, m, :], ew2[:, m, :],
                             start=(m == 0), stop=(m == 3))
        nc.vector.tensor_scalar_mul(outb[:], ps2[:], coef[:])
        nc.sync.dma_start(out_d[:], outb[:])
    nc.compile()
    return nc


# ---------------------------------------------------------------------------
# host orchestration
# ---------------------------------------------------------------------------

_CACHE = {}
LAST = {}
_TRACE = False


def set_trace(v):
    global _TRACE
    _TRACE = v


def _prep_conv_inputs(x, conv1_w, conv1_b, bn1_g, bn1_b, bn1_m, bn1_v,
                      conv2_w, conv2_b, bn2_g, bn2_b, bn2_m, bn2_v,
                      conv3_w, conv3_b, bn3_g, bn3_b, bn3_m, bn3_v):
    w1, bb1 = _fold_bn(conv1_w, conv1_b, bn1_g, bn1_b, bn1_m, bn1_v)
    w2, bb2 = _fold_bn(conv2_w, conv2_b, bn2_g, bn2_b, bn2_m, bn2_v)
    w3, bb3 = _fold_bn(conv3_w, conv3_b, bn3_g, bn3_b, bn3_m, bn3_v)
    shared = {
        "w1t": _l1_weights(w1), "w2t": _l2_weights(w2), "w3t": _l3_weights(w3),
        "b1": np.tile(bb1.astype(np.float32), 4)[:, None],
        "b2": np.tile(bb2.astype(np.float32), 2)[:, None],
        "b3": bb3.astype(np.float32)[:, None],
    }
    in_maps = []
    for c in range(NCORE):
        m = dict(shared)
        m["xblk"] = _l1_blocks(np.asarray(x[c * BL:(c + 1) * BL]))
        in_maps.append(m)
    return in_maps


def _route(feat, router_w, router_b):
    """Top-2 routing in float64. Returns coef [128, 8] (zeros off top-2)."""
    logits = feat.astype(np.float64) @ router_w.astype(np.float64).T \
        + router_b.astype(np.float64)
    order = np.argsort(-logits, axis=1, kind="stable")
    i0, i1 = order[:, 0], order[:, 1]
    t = np.arange(logits.shape[0])
    e1 = np.exp(logits[t, i1] - logits[t, i0])
    c0 = 1.0 / (1.0 + e1)
    coef = np.zeros((logits.shape[0], E), np.float64)
    coef[t, i0] = c0
    coef[t, i1] = 1.0 - c0
    return coef


import os as _os
_FUSED = _os.environ.get("KERNEL_FUSED", "1") == "1"


def kernel(**inputs):
    inputs = {k: np.asarray(v) for k, v in inputs.items()}
    if _FUSED:
        try:
            return kernel_fused(inputs)
        except Exception as e:  # fall back to the two-launch path
            import traceback
            traceback.print_exc()
            print(f"kernel_fused failed ({type(e).__name__}); "
                  "falling back to two-launch path", flush=True)
    conv_keys = ("x", "conv1_w", "conv1_b", "bn1_g", "bn1_b", "bn1_m", "bn1_v",
                 "conv2_w", "conv2_b", "bn2_g", "bn2_b", "bn2_m", "bn2_v",
                 "conv3_w", "conv3_b", "bn3_g", "bn3_b", "bn3_m", "bn3_v")
    in_maps = _prep_conv_inputs(*[inputs[k] for k in conv_keys])

    if "conv_nc" not in _CACHE:
        _CACHE["conv_nc"] = build_conv_module()
    res_a = run_bass_kernel_spmd(_CACHE["conv_nc"], in_maps,
                                 core_ids=list(range(NCORE)), trace=_TRACE)
    LAST["a"] = res_a

    # feat [128 tokens, 6272]; feat index f = c*49 + s
    feat = np.concatenate(
        [res_a.results[c]["feat"].transpose(1, 0, 2).reshape(BL, F)
         for c in range(NCORE)], axis=0).astype(np.float32)

    coef = _route(feat, inputs["router_w"], inputs["router_b"])

    featT = np.ascontiguousarray(
        feat.T.reshape(128, 49, 128).astype(BF16))          # [c, s, t]
    ew1 = inputs["ew1"]
    ew2 = inputs["ew2"]
    eb1 = inputs["eb1"]
    in_maps_b = []
    for e in range(NCORE):
        in_maps_b.append({
            "featT": featT,
            "ew1": np.ascontiguousarray(ew1[e].reshape(128, 49, 512).astype(BF16)),
            "ew2": np.ascontiguousarray(
                ew2[e].reshape(4, 128, 256).transpose(1, 0, 2).astype(BF16)),
            "eb1": np.ascontiguousarray(eb1[e].reshape(4, 128).T.astype(np.float32)),
            "coef": coef[:, e].astype(np.float32)[:, None],
        })
    if "exp_nc" not in _CACHE:
        _CACHE["exp_nc"] = build_expert_module()
    res_b = run_bass_kernel_spmd(_CACHE["exp_nc"], in_maps_b,
                                 core_ids=list(range(NCORE)), trace=_TRACE)
    LAST["b"] = res_b

    emb = sum(res_b.results[e]["partial"].astype(np.float64) for e in range(NCORE))
    emb += coef @ inputs["eb2"].astype(np.float64)           # eb2 correction
    norm = np.linalg.norm(emb, axis=1, keepdims=True)
    return (emb / np.maximum(norm, 1e-12)).astype(np.float32)


# ---------------------------------------------------------------------------
# fused single-launch module: conv -> AllGather(feat) -> router top-2 ->
# expert -> ReduceScatter -> L2 normalize
# ---------------------------------------------------------------------------

def build_fused_module():
    nc = bacc.Bacc("TRN2", target_bir_lowering=False, debug=False,
                   num_devices=NCORE)
    f32, bf = mybir.dt.float32, mybir.dt.bfloat16
    xblk_d = nc.dram_tensor("xblk", [36, 116, BL, 116], bf, kind="ExternalInput")
    w1_d = nc.dram_tensor("w1t", [96, 5, 3, 128], bf, kind="ExternalInput")
    w2_d = nc.dram_tensor("w2t", [96, 5, 2, 128], bf, kind="ExternalInput")
    w3_d = nc.dram_tensor("w3t", [110, 5, 3, 128], bf, kind="ExternalInput")
    b1_d = nc.dram_tensor("b1", [128, 1], f32, kind="ExternalInput")
    b2_d = nc.dram_tensor("b2", [128, 1], f32, kind="ExternalInput")
    b3_d = nc.dram_tensor("b3", [128, 1], f32, kind="ExternalInput")
    rwt_d = nc.dram_tensor("rwt", [128, 49, 8], f32, kind="ExternalInput")
    rb_d = nc.dram_tensor("rb", [1, 8], f32, kind="ExternalInput")
    ew1_d = nc.dram_tensor("ew1", [4, 128, 49, 128], bf, kind="ExternalInput")
    ew2_d = nc.dram_tensor("ew2", [128, 4, 256], bf, kind="ExternalInput")
    eb1_d = nc.dram_tensor("eb1", [128, 4], f32, kind="ExternalInput")
    eb2r_d = nc.dram_tensor("eb2r", [1, 256], bf, kind="ExternalInput")
    oh_d = nc.dram_tensor("oh", [128, 8], f32, kind="ExternalInput")
    y_d = nc.dram_tensor("y", [BL, 256], f32, kind="ExternalOutput")

    ag1_d = nc.dram_tensor("ag1", [128, 14, BL], bf, kind="Internal")
    cc1_d = nc.dram_tensor("cc1", [NCORE, 128, 14, BL], bf,
                           kind="Internal", addr_space="Shared")
    ag2_d = nc.dram_tensor("ag2", [128, 14, BL], bf, kind="Internal")
    cc2_d = nc.dram_tensor("cc2", [NCORE, 128, 14, BL], bf,
                           kind="Internal", addr_space="Shared")
    ag3_d = nc.dram_tensor("ag3", [128, 22, BL], bf, kind="Internal")
    cc3_d = nc.dram_tensor("cc3", [NCORE, 128, 22, BL], bf,
                           kind="Internal", addr_space="Shared")
    rs_in = nc.dram_tensor("rs_in", [128, 256], f32, kind="Internal")
    rs_out = nc.dram_tensor("rs_out", [BL, 256], f32, kind="Internal")
    RG = [list(range(NCORE))]

    from contextlib import ExitStack
    with tile.TileContext(nc) as tc, ExitStack() as ctx:
        wp = ctx.enter_context(tc.tile_pool(name="weights", bufs=1))
        xp_pool = ctx.enter_context(tc.tile_pool(name="xblk", bufs=2))
        blkp = ctx.enter_context(tc.tile_pool(name="blk", bufs=3))
        psp = ctx.enter_context(tc.tile_pool(name="psum", bufs=2, space="PSUM"))
        psq = ctx.enter_context(tc.tile_pool(name="psum2", bufs=1, space="PSUM"))
        vp = ctx.enter_context(tc.tile_pool(name="vtmp", bufs=4))
        pers = ctx.enter_context(tc.tile_pool(name="persist", bufs=1))

        w1t = wp.tile([96, 5, 3, 128], bf)
        w2t = wp.tile([96, 5, 2, 128], bf)
        w3t = wp.tile([110, 5, 3, 128], bf)
        b1t = wp.tile([128, 1], f32)
        b2t = wp.tile([128, 1], f32)
        b3t = wp.tile([128, 1], f32)
        for dst, src, q in ((w1t, w1_d, nc.sync), (b1t, b1_d, nc.sync),
                            (b2t, b2_d, nc.scalar), (b3t, b3_d, nc.scalar)):
            q.dma_start(dst[:], src[:])

        pool1 = pers.tile([128, 60, 4, 60], bf)
        pool2 = pers.tile([128, 32, 8, 32], bf)
        feat_sb = pers.tile([128, 49, BL], f32)   # s-major: s = 7*y + x
        fz = pers.tile([128, 50, BL], bf)         # bf16 AG payload, row 49 = logits
        # only the pad borders are ever read as zeros; memset them on DVE so
        # the gpsimd DMA queue isn't blocked at kernel start
        for t, n in ((pool1, 60), (pool2, 32)):
            nc.vector.memset(t[:, 0:2, :, :], 0.0)
            nc.vector.memset(t[:, n - 2:n, :, :], 0.0)
            nc.vector.memset(t[:, :, :, 0:2], 0.0)
            nc.vector.memset(t[:, :, :, n - 2:n], 0.0)

        # expert L1 weights: 16 quarter-loads pinned via dummy WAW deps
        # (tiny copy from a pool element written the block before) so the
        # scheduler cannot hoist the ~1.2us transfers into the input feed;
        # m=0,1 stream during late L1 (big DMA slack), m=2,3 during L2
        ew1m = []
        for m in range(4):
            w = wp.tile([128, 49, 128], bf, name=f"ew1m{m}")
            ew1m.append(w)
        EW1_Q = [0, 13, 25, 37, 49]
        EW1_SCHED_L1 = {20: (0, 0), 21: (0, 1), 22: (0, 2), 23: (0, 3),
                        24: (1, 0), 25: (1, 1), 26: (1, 2), 27: (1, 3)}

        # ---------------- conv stack (identical to build_conv_module)
        for b in range(28):
            xt = xp_pool.tile([96, 3, BL, 116], bf)
            qj = (nc.sync, nc.scalar, nc.gpsimd) if b == 0 else \
                 (nc.sync, nc.sync, nc.gpsimd)
            for j, q in zip(range(3), qj):
                q.dma_start(
                    xt[:, j].rearrange("p i x -> p (i x)"),
                    xblk_d[12 * j:12 * j + 12, 4 * b:4 * b + 8]
                    .rearrange("c y i x -> c y (i x)"))
            if b == 0:
                # w2t/w3t ride after the first input block so the DMA engines
                # serve L1's critical path first
                nc.scalar.dma_start(w2t[:], w2_d[:])
                nc.scalar.dma_start(w3t[:], w3_d[:])
            for sg in range(4):
                ps = psp.tile([128, 4, 112], mybir.dt.float32)
                k = 0
                for kx in range(5):
                    for j in range(3):
                        nc.tensor.matmul(
                            ps[:], w1t[:, kx, j, :],
                            xt[:, j, 4 * sg:4 * sg + 4, kx:kx + 112],
                            start=(k == 0), stop=(k == 14))
                        k += 1
                ev = vp.tile([128, 4, 112], bf)
                nc.scalar.activation(out=ev[:], in_=ps[:], func=RELU,
                                     bias=b1t[:], scale=1.0)
                xpo = vp.tile([32, 4, 4, 56], bf)
                for yo in range(4):
                    nc.vector.tensor_tensor(
                        xpo[:, yo], ev[32 * yo:32 * yo + 32, :, 0::2],
                        ev[32 * yo:32 * yo + 32, :, 1::2], MAX)
                for yy in range(2):
                    nc.vector.tensor_tensor(
                        pool1[32 * sg:32 * sg + 32, 2 + 2 * b + yy, :, 2:58],
                        xpo[:, 2 * yy], xpo[:, 2 * yy + 1], MAX)
            if b in EW1_SCHED_L1:
                m, hh = EW1_SCHED_L1[b]
                s0, s1 = EW1_Q[hh], EW1_Q[hh + 1]
                nc.vector.tensor_copy(
                    ew1m[m][0:1, s0:s0 + 1, 0:1].rearrange("c s h -> c (s h)"),
                    pool1[0:1, 2 * b:2 * b + 1, 0:1, 2:3]
                    .rearrange("c y i x -> c (y i x)"))
                nc.sync.dma_start(ew1m[m][:, s0:s1, :], ew1_d[m, :, s0:s1, :])

        EW1_SCHED = {4: (2, 0), 7: (2, 1), 10: (2, 2), 13: (2, 3),
                     16: (3, 0), 19: (3, 1), 22: (3, 2), 25: (3, 3)}
        L2_QUEUES = (nc.sync, nc.sync, nc.sync, nc.scalar,
                     nc.scalar, nc.gpsimd, nc.gpsimd, nc.gpsimd)
        for b in range(28):
            bt = blkp.tile([96, 2, 4, 4, 60], bf)
            for j in range(2):
                for q in range(4):
                    src = pool1[32 * q + 16 * j:32 * q + 16 * j + 16,
                                2 * b:2 * b + 6, :, :]
                    L2_QUEUES[4 * j + q].dma_start(
                        bt[:, j, q].rearrange("p i x -> p (i x)"),
                        src.rearrange("c y i x -> c y (i x)"))
            for sg in range(2):
                ps = psp.tile([128, 2, 4, 56], mybir.dt.float32)
                k = 0
                for kx in range(5):
                    for j in range(2):
                        nc.tensor.matmul(
                            ps[:], w2t[:, kx, j, :],
                            bt[:, j, 2 * sg:2 * sg + 2, :, kx:kx + 56],
                            start=(k == 0), stop=(k == 9))
                        k += 1
                ev = vp.tile([128, 2, 4, 56], bf, tag="ev2")
                nc.scalar.activation(out=ev[:], in_=ps[:], func=RELU,
                                     bias=b2t[:], scale=1.0)
                xpo = vp.tile([64, 2, 2, 4, 28], bf, tag="xpo2")
                for yo in range(2):
                    nc.vector.tensor_tensor(
                        xpo[:, yo], ev[64 * yo:64 * yo + 64, :, :, 0::2],
                        ev[64 * yo:64 * yo + 64, :, :, 1::2], MAX)
                nc.vector.tensor_tensor(
                    pool2[64 * sg:64 * sg + 64, 2 + b, :, 2:30]
                    .rearrange("c (q i) x -> c q i x", q=2),
                    xpo[:, 0], xpo[:, 1], MAX)
            if b in EW1_SCHED:
                m, hh = EW1_SCHED[b]
                s0, s1 = EW1_Q[hh], EW1_Q[hh + 1]
                nc.vector.tensor_copy(
                    ew1m[m][0:1, s0:s0 + 1, 0:1].rearrange("c s h -> c (s h)"),
                    pool2[0:1, 1 + b:2 + b, 0:1, 2:3]
                    .rearrange("c y i x -> c (y i x)"))
                nc.sync.dma_start(ew1m[m][:, s0:s1, :], ew1_d[m, :, s0:s1, :])

        # router weights + payload setup; logits accumulate into ps_r as
        # feat rows complete (interleaved two blocks late so PE never waits)
        rwt = wp.tile([128, 49, 8], f32)
        rbt = wp.tile([1, 8], f32)
        ones32 = wp.tile([1, BL], f32)
        ones = wp.tile([1, 128], bf)
        oht = wp.tile([128, 8], f32)
        nc.scalar.dma_start(rwt[:], rwt_d[:])
        nc.scalar.dma_start(rbt[:], rb_d[:])
        nc.scalar.dma_start(oht[:], oh_d[:])
        nc.vector.memset(ones[:], 1.0)
        nc.vector.memset(ones32[:], 1.0)
        nc.vector.memset(fz[:, 49, :], 0.0)
        ps_r = psq.tile([BL, 8], mybir.dt.float32, tag="psr")

        L3_QUEUES = (nc.sync, nc.gpsimd, nc.sync, nc.gpsimd, nc.sync, nc.scalar)
        L3_QUEUES_NOPOOL = (nc.sync, nc.scalar, nc.sync, nc.scalar,
                            nc.sync, nc.scalar)
        for b in range(28):
            # blocks right after an in-loop collective: keep Pool free so the
            # collective's sem wait can't stall the gather feed
            qs = L3_QUEUES_NOPOOL if b in (8, 9, 16, 17) else L3_QUEUES
            bt = blkp.tile([110, 3, 2, 8, 32], bf, tag="bt3")
            for j, (coff, ccnt) in enumerate(L3_CH):
                for h in range(2):
                    src = pool2[64 * h + coff:64 * h + coff + ccnt, b:b + 5, :, :]
                    qs[2 * j + h].dma_start(
                        bt[0:ccnt * 5, j, h].rearrange("p i x -> p (i x)"),
                        src.rearrange("c y i x -> c y (i x)"))
            ps = psp.tile([128, 2, 8, 28], mybir.dt.float32)
            k = 0
            for kx in range(5):
                for j, (coff, ccnt) in enumerate(L3_CH):
                    nc.tensor.matmul(
                        ps[:], w3t[0:ccnt * 5, kx, j, :],
                        bt[0:ccnt * 5, j, :, :, kx:kx + 28],
                        start=(k == 0), stop=(k == 14))
                    k += 1
            ev3 = vp.tile([128, BL, 28], mybir.dt.float32, tag="ev3", bufs=6)
            nc.scalar.activation(
                out=ev3[:], in_=ps[:].rearrange("c h i x -> c (h i) x"),
                func=RELU, bias=b3t[:], scale=1.0)
            # streamed maxpool 28->14 (x now, y across block pairs) and
            # avgpool 14->7 (x now, y across row pairs)
            ve = nc.vector
            xr = vp.tile([128, BL, 14], mybir.dt.float32,
                         tag=f"xr{b % 2}", bufs=3, name=f"xr_{b}")
            ve.tensor_tensor(xr[:], ev3[:, :, 0::2], ev3[:, :, 1::2], MAX)
            if b % 2 == 0:
                xr_prev = xr
            else:
                yp = (b - 1) // 2
                mrow = vp.tile([128, BL, 14], mybir.dt.float32, tag="mrow",
                               bufs=3, name=f"mrow_{b}")
                ve.tensor_tensor(mrow[:], xr_prev[:], xr[:], MAX)
                avx = vp.tile([128, BL, 7], mybir.dt.float32,
                              tag=f"av{yp % 2}", bufs=3, name=f"avx_{b}")
                ve.tensor_tensor(avx[:], mrow[:, :, 0::2],
                                 mrow[:, :, 1::2], ADD)
                if yp % 2 == 0:
                    avx_prev = avx
                else:
                    # feat rows stored as 4*avgpool (0.25 folded into rwt/ew1
                    # host-side); fz write is independent of feat_sb so the
                    # AG payload chain is 1 op shorter
                    yr = (yp - 1) // 2
                    ve.tensor_tensor(
                        fz[:, 7 * yr:7 * yr + 7, :]
                        .rearrange("c s t -> c t s"),
                        avx_prev[:], avx[:], ADD)
                    ve.tensor_tensor(
                        feat_sb[:, 7 * yr:7 * yr + 7, :]
                        .rearrange("c s t -> c t s"),
                        avx_prev[:], avx[:], ADD)
            if b >= 5 and (b - 5) % 4 == 0:
                yrr = (b - 5) // 4
                for s in range(7 * yrr, 7 * yrr + 7):
                    nc.tensor.matmul(ps_r[:], feat_sb[:, s, :], rwt[:, s, :],
                                     start=(s == 0), stop=False)
            if b == 7:
                # feat rows 0,1 (s 0..14) are final; AllGather chunk 1
                nc.gpsimd.dma_start(ag1_d[:], fz[:, 0:14, :])
                nc.gpsimd.collective_compute(
                    "AllGather", mybir.AluOpType.bypass, replica_groups=RG,
                    ins=[ag1_d[:]], outs=[cc1_d[:]])
            if b == 15:
                # feat rows 2,3 (s 14..28): AllGather chunk 2
                nc.gpsimd.dma_start(ag2_d[:], fz[:, 14:28, :])
                nc.gpsimd.collective_compute(
                    "AllGather", mybir.AluOpType.bypass, replica_groups=RG,
                    ins=[ag2_d[:]], outs=[cc2_d[:]])

        # ---------------- router tail + final AG chunk FIRST (critical path:
        # logits -> ag3 -> collective3), then staging/loads that overlap it
        for s in range(42, 49):
            nc.tensor.matmul(ps_r[:], feat_sb[:, s, :], rwt[:, s, :],
                             start=False, stop=False)
        nc.tensor.matmul(ps_r[:], ones32[:], rbt[:], start=False, stop=True)
        lgl = vp.tile([BL, 8], f32, tag="lgl", bufs=1)
        nc.vector.tensor_copy(lgl[:], ps_r[:])
        nc.vector.tensor_copy(fz[0:BL, 49, :], lgl[:].bitcast(bf))
        nc.scalar.dma_start(ag3_d[:], fz[:, 28:50, :])
        nc.gpsimd.collective_compute(
            "AllGather", mybir.AluOpType.bypass, replica_groups=RG,
            ins=[ag3_d[:]], outs=[cc3_d[:]])

        featT = pers.tile([128, NCORE, 49, BL], bf, tag="pool2")
        nc.sync.dma_start(featT[:, :, 0:14, :],
                          cc1_d[:].rearrange("r c s t -> c r s t"))
        nc.sync.dma_start(featT[:, :, 14:28, :],
                          cc2_d[:].rearrange("r c s t -> c r s t"))
        ew2t = wp.tile([128, 4, 256], bf)
        eb1t = wp.tile([128, 4], f32)
        eb2t = wp.tile([1, 256], bf)
        nc.sync.dma_start(ew2t[:], ew2_d[:])
        nc.scalar.dma_start(eb1t[:], eb1_d[:])
        nc.scalar.dma_start(eb2t[:], eb2r_d[:])
        # logits first so the softmax overlaps the chunk-3 feat staging
        lg = vp.tile([128, 8], f32, tag="lg")
        for r in range(NCORE):
            nc.sync.dma_start(
                lg[BL * r:BL * r + BL, :].bitcast(bf), cc3_d[r, 0:BL, 21, :])
        nc.scalar.dma_start(
            featT[:, :, 28:49, :],
            cc3_d[:, :, 0:21, :].rearrange("r c s t -> c r s t"))
        m1 = vp.tile([128, 1], f32, tag="m1")
        nc.vector.tensor_reduce(m1[:], lg[:], axis=mybir.AxisListType.X, op=MAX)
        negm1 = vp.tile([128, 1], f32, tag="negm1")
        nc.vector.tensor_scalar_mul(negm1[:], m1[:], -1.0)
        eqm = vp.tile([128, 8], f32, tag="eqm")
        nc.vector.tensor_scalar(eqm[:], lg[:], m1[:], None,
                                op0=mybir.AluOpType.is_equal)
        pen = vp.tile([128, 8], f32, tag="pen")
        nc.vector.tensor_scalar_mul(pen[:], eqm[:], -1e30)
        msk = vp.tile([128, 8], f32, tag="msk")
        nc.vector.tensor_tensor(msk[:], lg[:], pen[:], ADD)
        m2 = vp.tile([128, 1], f32, tag="m2")
        nc.vector.tensor_reduce(m2[:], msk[:], axis=mybir.AxisListType.X, op=MAX)
        z = vp.tile([128, 8], f32, tag="z")
        nc.scalar.activation(out=z[:], in_=lg[:],
                             func=mybir.ActivationFunctionType.Exp,
                             bias=negm1[:], scale=1.0)
        e2 = vp.tile([128, 1], f32, tag="e2")
        nc.scalar.activation(out=e2[:], in_=m2[:],
                             func=mybir.ActivationFunctionType.Exp,
                             bias=negm1[:], scale=1.0)
        den = vp.tile([128, 1], f32, tag="den")
        nc.vector.tensor_scalar_add(den[:], e2[:], 1.0)
        rden = vp.tile([128, 1], f32, tag="rden")
        nc.vector.reciprocal(rden[:], den[:])
        g = vp.tile([128, 8], f32, tag="g")
        nc.vector.tensor_scalar(g[:], lg[:], m2[:], None,
                                op0=mybir.AluOpType.is_ge)
        zg = vp.tile([128, 8], f32, tag="zg")
        nc.vector.tensor_tensor(zg[:], z[:], g[:], mybir.AluOpType.mult)
        coef = vp.tile([128, 8], f32, tag="coef")
        nc.vector.tensor_scalar_mul(coef[:], zg[:], rden[:])
        cm = vp.tile([128, 8], f32, tag="cm")
        nc.vector.tensor_tensor(cm[:], coef[:], oht[:], mybir.AluOpType.mult)
        coefe = vp.tile([128, 1], f32, tag="coefe")
        nc.vector.tensor_reduce(coefe[:], cm[:], axis=mybir.AxisListType.X, op=ADD)

        # ---------------- expert MLP: s-outer so chunk-1 rows (s<28) start
        # before chunk 2 lands; 4 live PSUM accumulators
        psm = [psq.tile([128, 128], mybir.dt.float32, tag=f"psm{m}",
                        name=f"psm{m}")
               for m in range(4)]
        for s in range(49):
            for m in range(4):
                nc.tensor.matmul(psm[m][:], ew1m[m][:, s, :], featT[:, :, s, :],
                                 start=(s == 0), stop=(s == 48))
        hid = pers.tile([128, 4, 128], bf)
        for m in range(4):
            nc.scalar.activation(out=hid[:, m, :], in_=psm[m][:], func=RELU,
                                 bias=eb1t[:, m:m + 1], scale=1.0)
        ps2 = psq.tile([128, 256], mybir.dt.float32, tag="ps2")
        for m in range(4):
            nc.tensor.matmul(ps2[:], hid[:, m, :], ew2t[:, m, :],
                             start=(m == 0), stop=False)
        nc.tensor.matmul(ps2[:], ones[:], eb2t[:], start=False, stop=True)
        wout = vp.tile([128, 256], f32, tag="wout", bufs=1)
        nc.vector.tensor_scalar_mul(wout[:], ps2[:], coefe[:])
        nc.sync.dma_start(rs_in[:], wout[:])

        # ---------------- ReduceScatter (sum over experts, keep token shard)
        nc.gpsimd.collective_compute(
            "ReduceScatter", mybir.AluOpType.add, replica_groups=RG,
            ins=[rs_in[:]], outs=[rs_out[:]])

        # ---------------- L2 normalize token shard
        nsb = vp.tile([BL, 256], f32, tag="nsb", bufs=1)
        nc.sync.dma_start(nsb[:], rs_out[:])
        sq = vp.tile([BL, 256], f32, tag="sq", bufs=1)
        nc.vector.tensor_tensor(sq[:], nsb[:], nsb[:], mybir.AluOpType.mult)
        ss = vp.tile([BL, 1], f32, tag="ss")
        nc.vector.tensor_reduce(ss[:], sq[:], axis=mybir.AxisListType.X, op=ADD)
        nrm = vp.tile([BL, 1], f32, tag="nrm")
        nc.scalar.activation(out=nrm[:], in_=ss[:],
                             func=mybir.ActivationFunctionType.Sqrt, scale=1.0)
        nc.vector.tensor_scalar_max(nrm[:], nrm[:], 1e-12)
        rn = vp.tile([BL, 1], f32, tag="rn")
        nc.vector.reciprocal(rn[:], nrm[:])
        yt = vp.tile([BL, 256], f32, tag="yt", bufs=1)
        nc.vector.tensor_scalar_mul(yt[:], nsb[:], rn[:])
        nc.sync.dma_start(y_d[:], yt[:])
    nc.compile()
    return nc


def kernel_fused(inputs):
    conv_keys = ("x", "conv1_w", "conv1_b", "bn1_g", "bn1_b", "bn1_m", "bn1_v",
                 "conv2_w", "conv2_b", "bn2_g", "bn2_b", "bn2_m", "bn2_v",
                 "conv3_w", "conv3_b", "bn3_g", "bn3_b", "bn3_m", "bn3_v")
    in_maps = _prep_conv_inputs(*[inputs[k] for k in conv_keys])
    rw = inputs["router_w"]
    shared = {
        "rwt": np.ascontiguousarray(
            (0.25 * rw.T).reshape(128, 49, 8).astype(np.float32)),
        "rb": inputs["router_b"].astype(np.float32)[None, :],
    }
    for e in range(NCORE):
        oh = np.zeros((128, 8), np.float32)
        oh[:, e] = 1.0
        in_maps[e].update(shared)
        in_maps[e]["ew1"] = np.ascontiguousarray(
            (0.25 * inputs["ew1"][e]).reshape(128, 49, 4, 128)
            .transpose(2, 0, 1, 3).astype(BF16))
        in_maps[e]["ew2"] = np.ascontiguousarray(
            inputs["ew2"][e].reshape(4, 128, 256).transpose(1, 0, 2).astype(BF16))
        in_maps[e]["eb1"] = np.ascontiguousarray(
            inputs["eb1"][e].reshape(4, 128).T.astype(np.float32))
        in_maps[e]["eb2r"] = inputs["eb2"][e].astype(BF16)[None, :]
        in_maps[e]["oh"] = oh
    if "fused_nc" not in _CACHE:
        _CACHE["fused_nc"] = build_fused_module()
    res = run_bass_kernel_spmd(_CACHE["fused_nc"], in_maps,
                               core_ids=list(range(NCORE)))
    LAST["a"] = res
    LAST.pop("b", None)
    return np.concatenate([res.results[c]["y"] for c in range(NCORE)], axis=0)

